# revision 1
# baseline (speedup 1.0000x reference)
"""Two-layer GCN (PyG GCNConv semantics) on 8 Trainium2 NeuronCores.

Strategy (sharding_hint): nodes are sharded row-wise across the 8 cores;
edges are partitioned by destination node so the segment-sum stays local;
source-node features are exchanged with an on-device AllGather between
layers; the small weight matrices are replicated.

Gather pipeline: per-edge source rows are fetched with batched SWDGE
dma_gather (int16 indices, thousands of rows per instruction) instead of
per-128-row indirect DMAs — this takes the Pool-engine descriptor
generation off the critical path.  Because dma_gather indices are int16,
the gather table is processed in 4 ranges of 32768 rows; edges are grouped
host-side by (destination window, source range) and padded to 128-slot
tiles so every tile is single-window and single-range.

Aggregation per 128-node destination window: a norm-weighted one-hot
S[e, j] = norm[e] * (dst_rel[e] == j) built in one DVE tensor_scalar per
tile, then PE matmuls accumulate msgs into PSUM.  Epilogues: relu+bias
into an SBUF-resident fp16 transposed activation accumulator (layer 1);
bias + log_softmax (layer 2).

Edge bookkeeping (sorting, slot assignment, padding so all 8 cores share
one instruction stream) is host-side numpy index work; all floating-point
math on features runs on device.
"""

import math

import numpy as np

import concourse.bass as bass
import concourse.mybir as mybir
import concourse.tile as tile
from concourse import library_config
from concourse.bass_utils import run_bass_kernel_spmd

N_NODES = 100000
N_EDGES = 1600000
IN_DIM, HID_DIM, OUT_DIM = 128, 64, 40
N_CORES = 8

RNG = 25000          # rows per gather range (must fit int16 indices)
NR = 4               # number of ranges covering N_NODES
CHUNK = 32           # tiles per dma_gather instruction
GBUFS = 10           # gather chunk buffers in flight
SBUFS = 20

# diagnostics (timing bisection only; wrong results when enabled)
NO_S = False
NO_MM = False
NO_GATHER = False
NO_AG = False
NO_EPI = False
PBUFS = 4

F32 = mybir.dt.float32
F16 = mybir.dt.float16
I16 = mybir.dt.int16


def _split_long_waits(nc, max_waits=1):
    """This toolchain's codegen rejects instructions carrying more than one
    semaphore wait; move extra waits onto preceding same-engine no-ops."""
    cnt = 0
    for bb in nc.main_func.blocks:
        i = 0
        insts = bb.instructions
        while i < len(insts):
            ins = insts[i]
            si = ins.sync_info
            if si is not None and si.on_wait and len(si.on_wait) > max_waits:
                waits = list(si.on_wait)
                keep = waits[-max_waits:]
                extra = waits[:-max_waits]
                si.on_wait = keep
                new_insts = []
                for j in range(0, len(extra), max_waits):
                    chunk = extra[j : j + max_waits]
                    nop = mybir.InstNoOp(
                        name=f"{ins.name}-waitsplit-{j}",
                        engine=ins.engine,
                        ins=[],
                        outs=[],
                        sync_info=mybir.SyncInfo(on_wait=chunk, on_update=[]),
                    )
                    new_insts.append(nop)
                insts[i:i] = new_insts
                i += len(new_insts)
                cnt += len(new_insts)
            i += 1
    return cnt


def _preprocess(edge_index, n_nodes, n_cores):
    """Host-side index bookkeeping. Returns per-core slot arrays + layout."""
    nloc = n_nodes // n_cores
    wn = math.ceil(nloc / 128)

    src = np.asarray(edge_index[0], dtype=np.int64)
    dst = np.asarray(edge_index[1], dtype=np.int64)
    loop = np.arange(n_nodes, dtype=np.int64)
    src_all = np.concatenate([src, loop])
    dst_all = np.concatenate([dst, loop])

    deg = np.bincount(dst_all, minlength=n_nodes).astype(np.float64)
    dis = np.where(deg > 0, 1.0 / np.sqrt(deg), 0.0)
    norm = (dis[src_all] * dis[dst_all]).astype(np.float32)

    core = dst_all // nloc
    dloc = dst_all - core * nloc
    w = dloc >> 7
    drel = (dloc & 127).astype(np.float32)
    r = src_all // RNG
    assert r.max() < NR

    key = (core * wn + w) * NR + r
    order = np.argsort(key, kind="stable")
    counts = np.bincount(key, minlength=n_cores * wn * NR).reshape(n_cores, wn, NR)

    # tiles per (window, range): shared across cores (single SPMD program)
    k_wr = np.maximum((counts + 127) // 128, 0).max(axis=0)  # [wn, NR]
    k_wr[:, 0] = np.maximum(k_wr[:, 0], 1)  # every window needs >=1 tile

    T_r = k_wr.sum(axis=0)  # tiles per range  [NR]
    t_start = np.zeros(NR + 1, dtype=np.int64)
    t_start[1:] = np.cumsum(T_r)
    t_total = int(t_start[NR])

    # global tile id of the first tile of (w, r): r-major layout
    trw = np.zeros((NR, wn), dtype=np.int64)
    for rr in range(NR):
        trw[rr, 0] = t_start[rr]
        trw[rr, 1:] = t_start[rr] + np.cumsum(k_wr[:, rr])[:-1]

    # rank of each edge within its (core, w, r) group, in sorted order
    grp_start = np.zeros(n_cores * wn * NR, dtype=np.int64)
    grp_start[1:] = np.cumsum(counts.reshape(-1))[:-1]
    rank = np.arange(len(order), dtype=np.int64) - grp_start[key[order]]

    w_o = w[order]
    r_o = r[order]
    slot = (trw[r_o, w_o] + (rank >> 7)) * 128 + (rank & 127)

    n_slots = t_total * 128
    idx_a = np.zeros((n_cores, n_slots), dtype=np.int16)
    drel_a = np.zeros((n_cores, n_slots), dtype=np.float32)
    nrm_a = np.zeros((n_cores, n_slots), dtype=np.float32)

    c_o = core[order]
    idx_a[c_o, slot] = (src_all[order] - r_o * RNG).astype(np.int16)
    drel_a[c_o, slot] = drel[order]
    nrm_a[c_o, slot] = norm[order]

    # device layouts:
    #   idx: [128, t_total*8] int16, slot s at [16g + s%16, s//16], g=0..7
    #   drel/nrm: [128, t_total] f32, slot s at [s%128, s//128]
    idx_dev, drel_dev, nrm_dev = [], [], []
    for c in range(n_cores):
        base = idx_a[c].reshape(n_slots // 16, 16).T  # [16, t_total*8]
        idx_dev.append(np.tile(base, (8, 1)).copy())
        drel_dev.append(drel_a[c].reshape(t_total, 128).T.copy())
        nrm_dev.append(nrm_a[c].reshape(t_total, 128).T.copy())

    return {
        "nloc": nloc,
        "wn": wn,
        "k_wr": k_wr,
        "T_r": [int(v) for v in T_r],
        "t_start": [int(v) for v in t_start],
        "trw": trw,
        "t_total": t_total,
        "idx": idx_dev,
        "drel": drel_dev,
        "norm": nrm_dev,
    }


def _build_nc(meta, n_nodes, hid, out_dim, in_dim, n_cores, rounds=1):
    nloc = meta["nloc"]
    wn = meta["wn"]
    k_wr = meta["k_wr"]
    T_r = meta["T_r"]
    t_start = meta["t_start"]
    trw = meta["trw"]
    t_total = meta["t_total"]

    nc = bass.Bass(num_devices=n_cores, num_swdge_queues=4)

    xT16 = nc.dram_tensor("xT16", [in_dim, nloc], F16, kind="ExternalInput")
    idx = nc.dram_tensor("idx", [128, t_total * 8], I16, kind="ExternalInput")
    drel = nc.dram_tensor("drel", [128, t_total], F32, kind="ExternalInput")
    nrm = nc.dram_tensor("nrm", [128, t_total], F32, kind="ExternalInput")
    nneg = nc.dram_tensor("nneg", [128, t_total], F32, kind="ExternalInput")
    w1 = nc.dram_tensor("w1", [in_dim, hid], F16, kind="ExternalInput")
    w2p = nc.dram_tensor("w2p", [hid, hid], F16, kind="ExternalInput")
    b1c = nc.dram_tensor("b1c", [hid, 1], F32, kind="ExternalInput")
    b2rep = nc.dram_tensor("b2rep", [128, hid], F32, kind="ExternalInput")
    iota_in = nc.dram_tensor("iota", [128, 128], F16, kind="ExternalInput")
    out = nc.dram_tensor("out", [nloc, out_dim], F32, kind="ExternalOutput")

    nb = math.ceil(nloc / 128)
    eq = mybir.AluOpType.is_equal
    mul = mybir.AluOpType.mult

    with tile.TileContext(nc) as tc:
        with (
            tc.tile_pool(name="const", bufs=1) as cp,
            tc.tile_pool(name="gpool", bufs=GBUFS) as gp,
            tc.tile_pool(name="spool", bufs=SBUFS) as sp,
            tc.tile_pool(name="evac", bufs=4) as ep,
            tc.tile_pool(name="ps_agg", bufs=PBUFS, space="PSUM") as pa,
            tc.tile_pool(name="ps_mm", bufs=3, space="PSUM") as pm,
            tc.tile_pool(name="ps_const", bufs=1, space="PSUM") as pc,
            tc.tile_pool(name="dram", bufs=1, space="DRAM") as dp,
        ):
            nc.gpsimd.load_library(library_config.mlp)

            # ---- resident tensors ----
            xT_t = cp.tile([in_dim, nloc], F16)
            nc.sync.dma_start(out=xT_t[:], in_=xT16[:])
            idx_t = cp.tile([128, t_total * 8], I16)
            nc.sync.dma_start(out=idx_t[:], in_=idx[:])
            drel_t = cp.tile([128, t_total], F32)
            nc.sync.dma_start(out=drel_t[:], in_=drel[:])
            nrm_t = cp.tile([128, t_total], F32)
            nc.sync.dma_start(out=nrm_t[:], in_=nrm[:])
            nneg_t = cp.tile([128, t_total], F32)
            nc.sync.dma_start(out=nneg_t[:], in_=nneg[:])
            w1_t = cp.tile([in_dim, hid], F16)
            nc.sync.dma_start(out=w1_t[:], in_=w1[:])
            w2_t = cp.tile([hid, hid], F16)
            nc.sync.dma_start(out=w2_t[:], in_=w2p[:])
            b1_t = cp.tile([hid, 1], F32)
            nc.sync.dma_start(out=b1_t[:], in_=b1c[:])
            b2_t = cp.tile([128, hid], F32)
            nc.sync.dma_start(out=b2_t[:], in_=b2rep[:])
            iota_t = cp.tile([128, 128], F16)
            nc.sync.dma_start(out=iota_t[:], in_=iota_in[:])
            # iota in PSUM: a non-SBUF operand keeps the DVE S-builds out of
            # 2-port perf mode, which would lock GPSIMD (SWDGE descriptor
            # rings) out of SBUF and stall the gather pipeline.
            iota_ps = pc.tile([128, 128], F32)
            nc.scalar.activation(
                out=iota_ps[:], in_=iota_t[:],
                func=mybir.ActivationFunctionType.Identity,
            )
            acc1T = cp.tile([hid, wn * 128], F16)
            s_const = cp.tile([128, 128], F16)
            nc.vector.memset(s_const[:], 0.0)

            h1loc = dp.tile([nloc, 2 * hid], F16)
            h2loc = dp.tile([nloc, 2 * hid], F16)

            def build_s(t):
                """S[e, j] = norm[e] * (dst_rel[e] == j)."""
                if NO_S:
                    return s_const
                s = sp.tile([128, 128], F16, tag="s", name="s")
                if t % 6 == 5:
                    # ACT path: relu(norm - norm*(drel-iota)^2) == norm iff eq
                    tmp = sp.tile([128, 128], F16, tag="stmp", name="stmp")
                    nc.scalar.activation(
                        out=tmp[:], in_=iota_t[:],
                        func=mybir.ActivationFunctionType.Square,
                        bias=drel_t[:, t : t + 1], scale=-1.0,
                    )
                    nc.scalar.activation(
                        out=s[:], in_=tmp[:],
                        func=mybir.ActivationFunctionType.Relu,
                        bias=nrm_t[:, t : t + 1], scale=nneg_t[:, t : t + 1],
                    )
                else:
                    nc.vector.tensor_scalar(
                        out=s[:], in0=iota_ps[:],
                        scalar1=drel_t[:, t : t + 1],
                        scalar2=nrm_t[:, t : t + 1],
                        op0=eq, op1=mul,
                    )
                return s

            def pre_matmul(lhsT_t, w_t, dst_dram):
                """h = act_prev @ W for the local shard -> DRAM table."""
                for b in range(nb):
                    cols = min(128, nloc - b * 128)
                    ps = pm.tile([128, hid], F32, tag="pmm")
                    nc.tensor.matmul(
                        out=ps[:cols, :],
                        lhsT=lhsT_t[:, b * 128 : b * 128 + cols],
                        rhs=w_t[:],
                        start=True,
                        stop=True,
                    )
                    hb = ep.tile([128, hid], F16, tag="hb")
                    nc.vector.tensor_copy(out=hb[:cols, :], in_=ps[:cols, :])
                    nc.sync.dma_start(
                        out=dst_dram[b * 128 : b * 128 + cols, 0:hid],
                        in_=hb[:cols, :],
                    )

            def all_gather(src_dram, dst_dram):
                if NO_AG:
                    return
                nc.gpsimd.collective_compute(
                    "AllGather",
                    mybir.AluOpType.bypass,
                    replica_groups=[list(range(n_cores))],
                    ins=[src_dram[:].opt()],
                    outs=[dst_dram[0:n_nodes, :].opt()],
                )

            nidx_regs = {}

            def nidx_reg(n):
                if n not in nidx_regs:
                    nidx_regs[n] = nc.gpsimd.to_reg(n)
                return nidx_regs[n]

            def agg_pass(table, layer):
                """Chunk-pipelined gather + per-window aggregation."""
                issued = [0] * NR          # tiles issued per range
                chunks = [[] for _ in range(NR)]  # (tile_ref, t0, span)

                def ensure(rr, tiles_needed):
                    while issued[rr] < tiles_needed:
                        t0 = t_start[rr] + issued[rr]
                        span = min(CHUNK, T_r[rr] - issued[rr])
                        g = gp.tile([128, CHUNK, 2 * hid], F16, tag="g", name="g")
                        rows0 = rr * RNG
                        rows1 = min(rows0 + RNG, n_nodes)
                        if NO_GATHER:
                            chunks[rr].append((g, t0, span))
                            issued[rr] += span
                            continue
                        nc.gpsimd.dma_gather(
                            g[:, 0:span, :],
                            table[rows0:rows1, :],
                            idx_t[:, t0 * 8 : (t0 + span) * 8],
                            span * 128,
                            nidx_reg(span * 128),
                            2 * hid,
                            single_packet=False,
                            queue_num=rr,
                        )
                        chunks[rr].append((g, t0, span))
                        issued[rr] += span

                def gview(rr, t):
                    while True:
                        g, t0, span = chunks[rr][0]
                        if t < t0 + span:
                            return g[:, t - t0, 0:hid]
                        chunks[rr].pop(0)

                for w in range(wn):
                    kw = [int(k_wr[w][rr]) for rr in range(NR)]
                    ktot = sum(kw)
                    for rr in range(NR):
                        if kw[rr]:
                            ensure(rr, trw[rr][w] - t_start[rr] + kw[rr])
                    if layer == 1:
                        pw = pa.tile([hid, 128], F32, tag="pagg")
                    else:
                        pw = pa.tile([128, hid], F32, tag="pagg")
                    ki = 0
                    for rr in range(NR):
                        for k in range(kw[rr]):
                            t = int(trw[rr][w]) + k
                            gv = gview(rr, t)
                            s = build_s(t)
                            if NO_MM:
                                pass
                            elif layer == 1:
                                nc.tensor.matmul(
                                    out=pw[:], lhsT=gv, rhs=s[:],
                                    start=(ki == 0), stop=(ki == ktot - 1),
                                )
                            else:
                                nc.tensor.matmul(
                                    out=pw[:], lhsT=s[:], rhs=gv,
                                    start=(ki == 0), stop=(ki == ktot - 1),
                                )
                            ki += 1
                    if NO_EPI:
                        continue
                    if layer == 1:
                        # epilogue: acc1T[:, w*128:] = relu(pw + b1)  (fp16)
                        nc.scalar.activation(
                            out=acc1T[:, w * 128 : (w + 1) * 128],
                            in_=pw[:],
                            func=mybir.ActivationFunctionType.Relu,
                            bias=b1_t[:],
                        )
                    else:
                        rows = min(128, nloc - w * 128)
                        zt = ep.tile([128, hid], F32, tag="zt")
                        nc.vector.tensor_tensor(
                            out=zt[:], in0=pw[:], in1=b2_t[:], op=mybir.AluOpType.add
                        )
                        mx = ep.tile([128, 1], F32, tag="mx")
                        nc.vector.reduce_max(
                            mx[:], zt[:, :out_dim], axis=mybir.AxisListType.X
                        )
                        sh = ep.tile([128, out_dim], F32, tag="sh")
                        nc.vector.tensor_tensor(
                            out=sh[:], in0=zt[:, :out_dim],
                            in1=mx[:].broadcast_to([128, out_dim]),
                            op=mybir.AluOpType.subtract,
                        )
                        ex = ep.tile([128, out_dim], F32, tag="ex")
                        sm = ep.tile([128, 1], F32, tag="sm")
                        nc.scalar.activation(
                            out=ex[:], in_=sh[:],
                            func=mybir.ActivationFunctionType.Exp,
                            accum_out=sm[:],
                        )
                        lnt = ep.tile([128, 1], F32, tag="lnt")
                        nc.scalar.activation(
                            out=lnt[:], in_=sm[:],
                            func=mybir.ActivationFunctionType.Ln,
                        )
                        res = ep.tile([128, out_dim], F32, tag="res")
                        nc.vector.tensor_tensor(
                            out=res[:], in0=sh[:],
                            in1=lnt[:].broadcast_to([128, out_dim]),
                            op=mybir.AluOpType.subtract,
                        )
                        nc.sync.dma_start(
                            out=out[w * 128 : w * 128 + rows, :], in_=res[:rows, :]
                        )

            # ---- pipeline ----
            for rnd in range(rounds):
                h1full = dp.tile(
                    [n_nodes, 2 * hid], F16, addr_space="Shared",
                    tag=f"h1full{rnd}", name=f"h1full{rnd}",
                )
                h2full = dp.tile(
                    [n_nodes, 2 * hid], F16, addr_space="Shared",
                    tag=f"h2full{rnd}", name=f"h2full{rnd}",
                )
                pre_matmul(xT_t, w1_t, h1loc)
                all_gather(h1loc, h1full)
                agg_pass(h1full, layer=1)
                pre_matmul(acc1T, w2_t, h2loc)
                all_gather(h2loc, h2full)
                agg_pass(h2full, layer=2)

    _split_long_waits(nc)
    mybir.codegen_inst_isa_subclasses(nc)
    return nc


def _prepare(x, edge_index, W1, b1, W2, b2, n_nodes=N_NODES, n_cores=N_CORES):
    x = np.asarray(x, dtype=np.float32)
    W1 = np.asarray(W1, dtype=np.float32)
    b1 = np.asarray(b1, dtype=np.float32)
    W2 = np.asarray(W2, dtype=np.float32)
    b2 = np.asarray(b2, dtype=np.float32)

    in_dim = x.shape[1]
    hid = W1.shape[1]
    out_dim = W2.shape[1]

    meta = _preprocess(edge_index, n_nodes, n_cores)
    nloc = meta["nloc"]

    nc = _build_nc(meta, n_nodes, hid, out_dim, in_dim, n_cores)

    w2pad = np.zeros((hid, hid), dtype=np.float16)
    w2pad[:, :out_dim] = W2.astype(np.float16)
    b2p = np.zeros((hid,), dtype=np.float32)
    b2p[:out_dim] = b2
    b2rep = np.tile(b2p[None, :], (128, 1)).copy()
    b1c = b1.reshape(hid, 1).copy()
    iota = np.tile(np.arange(128, dtype=np.float16)[None, :], (128, 1)).copy()
    w1h = W1.astype(np.float16)

    in_maps = []
    for c in range(n_cores):
        xs = x[c * nloc : (c + 1) * nloc]
        in_maps.append(
            {
                "xT16": np.ascontiguousarray(xs.T.astype(np.float16)),
                "idx": meta["idx"][c],
                "drel": meta["drel"][c],
                "nrm": meta["norm"][c],
                "nneg": -meta["norm"][c],
                "w1": w1h,
                "w2p": w2pad,
                "b1c": b1c,
                "b2rep": b2rep,
                "iota": iota,
            }
        )
    return nc, in_maps


def kernel(x, edge_index, W1, b1, W2, b2):
    nc, in_maps = _prepare(x, edge_index, W1, b1, W2, b2)
    res = run_bass_kernel_spmd(nc, in_maps, core_ids=list(range(N_CORES)))
    return np.concatenate([res.results[c]["out"] for c in range(N_CORES)], axis=0)



# revision 10
# speedup vs baseline: 1.0668x; 1.0668x over previous
"""Two-layer GCN (PyG GCNConv semantics) on 8 Trainium2 NeuronCores.

Strategy: nodes are sharded row-wise across the 8 cores; edges are
partitioned by destination node so the segment-sum stays local.

Layer 1 gathers raw x rows (128 f16 = 256B, the SWDGE minimum, fully
used) directly from a replicated DRAM table and folds W1 in after
aggregation: PT[in, dst] = sum_e x[src_e] * S, then acc1 = relu(PT.T @ W1
+ b1).  This removes both the first pre-matmul and the first (25.6MB)
AllGather from the critical path; layer-1 gathers start at t=0.

Layer 2 all-gathers the compact acc1 [nloc, 64] f16 (12.8MB full) viewed
as a [50000, 128] pair table (256B rows hold two nodes).  Edges are
sorted by (dst window, pair range, src parity) so each 128-edge tile
statically consumes one 64-wide half of its gathered rows; W2 is folded
in per window after aggregation, then bias + log_softmax.

Gather pipeline: batched SWDGE dma_gather (int16 indices, CHUNK*128 rows
per instruction), one queue per index range.  Aggregation per 128-node
destination window: a norm-weighted one-hot S[e, j] = norm[e] *
(dst_rel[e] == j) built in one DVE tensor_scalar per tile (1 in 6 on the
scalar engine), PE matmuls accumulate into PSUM.

Edge bookkeeping (sorting, slot assignment, padding so all 8 cores share
one instruction stream) is host-side numpy index work; all floating-point
math on features runs on device.
"""

import math

import numpy as np

import concourse.bass as bass
import concourse.mybir as mybir
import concourse.tile as tile
from concourse import library_config
from concourse.bass_utils import run_bass_kernel_spmd

N_NODES = 100000
N_EDGES = 1600000
IN_DIM, HID_DIM, OUT_DIM = 128, 64, 40
N_CORES = 8

RNG = 25000          # table rows per gather range (int16 index limit)
CHUNK = 32           # tiles per dma_gather instruction
GBUFS = 7            # gather chunk buffers in flight
SBUFS = 16
ACT_NTH = 6          # every ACT_NTH-th S-build goes to the scalar engine

F32 = mybir.dt.float32
F16 = mybir.dt.float16
I16 = mybir.dt.int16


def _split_long_waits(nc, max_waits=1):
    """This toolchain's codegen rejects instructions carrying more than one
    semaphore wait; move extra waits onto preceding same-engine no-ops."""
    cnt = 0
    for bb in nc.main_func.blocks:
        i = 0
        insts = bb.instructions
        while i < len(insts):
            ins = insts[i]
            si = ins.sync_info
            if si is not None and si.on_wait and len(si.on_wait) > max_waits:
                waits = list(si.on_wait)
                keep = waits[-max_waits:]
                extra = waits[:-max_waits]
                si.on_wait = keep
                new_insts = []
                for j in range(0, len(extra), max_waits):
                    chunk = extra[j : j + max_waits]
                    nop = mybir.InstNoOp(
                        name=f"{ins.name}-waitsplit-{j}",
                        engine=ins.engine,
                        ins=[],
                        outs=[],
                        sync_info=mybir.SyncInfo(on_wait=chunk, on_update=[]),
                    )
                    new_insts.append(nop)
                insts[i:i] = new_insts
                i += len(new_insts)
                cnt += len(new_insts)
            i += 1
    return cnt


def _mkstream(n_cores, wn, core, w, gsub, n_gsub, queue_of_gsub, nq,
              idxval, drel, nrm):
    """Group edges by (core, dst window, subgroup); lay out 128-slot tiles
    shared across cores (single SPMD program).  Tile ids are queue-major;
    within a queue they follow consumption order (w, then subgroup, then
    k), so per-queue gathers are contiguous slices.

    Returns device-layout index/scalar arrays plus the tile layout.
    """
    key = (core.astype(np.int64) * wn + w) * n_gsub + gsub
    order = np.argsort(key, kind="stable")
    counts = np.bincount(key, minlength=n_cores * wn * n_gsub).reshape(
        n_cores, wn, n_gsub
    )
    k_wg = (counts.max(axis=0) + 127) // 128  # [wn, n_gsub]

    gs_of_q = [[g for g in range(n_gsub) if queue_of_gsub[g] == q]
               for q in range(nq)]
    tile_first = np.zeros((wn, n_gsub), dtype=np.int64)
    t = 0
    t_start = []
    for q in range(nq):
        t_start.append(t)
        for ww in range(wn):
            for g in gs_of_q[q]:
                tile_first[ww, g] = t
                t += int(k_wg[ww, g])
    T = t
    t_start.append(T)

    grp_start = np.zeros(n_cores * wn * n_gsub, dtype=np.int64)
    grp_start[1:] = np.cumsum(counts.reshape(-1))[:-1]
    rank = np.arange(len(order), dtype=np.int64) - grp_start[key[order]]

    w_o = w[order]
    g_o = gsub[order]
    slot = (tile_first[w_o, g_o] + (rank >> 7)) * 128 + (rank & 127)

    n_slots = T * 128
    idx_a = np.zeros((n_cores, n_slots), dtype=np.int16)
    drel_a = np.zeros((n_cores, n_slots), dtype=np.float32)
    nrm_a = np.zeros((n_cores, n_slots), dtype=np.float32)
    c_o = core[order]
    idx_a[c_o, slot] = idxval[order].astype(np.int16)
    drel_a[c_o, slot] = drel[order]
    nrm_a[c_o, slot] = nrm[order]

    # device layouts:
    #   idx: [128, T*8] int16, slot s at [16g + s%16, s//16], g=0..7
    #   drel/nrm: [128, T] f32, slot s at [s%128, s//128]
    idx_dev, drel_dev, nrm_dev = [], [], []
    for c in range(n_cores):
        base = idx_a[c].reshape(n_slots // 16, 16).T
        idx_dev.append(np.tile(base, (8, 1)).copy())
        drel_dev.append(drel_a[c].reshape(T, 128).T.copy())
        nrm_dev.append(nrm_a[c].reshape(T, 128).T.copy())

    return {
        "T": T,
        "t_start": t_start,
        "nq": nq,
        "k_wg": k_wg,
        "tile_first": tile_first,
        "gs_of_q": gs_of_q,
        "queue_of_gsub": list(queue_of_gsub),
        "idx": idx_dev,
        "drel": drel_dev,
        "nrm": nrm_dev,
        # host-side slot arrays for preprocessing verification
        "host_slot": (c_o, slot, order),
    }


def _preprocess(edge_index, n_nodes, n_cores):
    nloc = n_nodes // n_cores
    wn = math.ceil(nloc / 128)

    src = np.asarray(edge_index[0], dtype=np.int64)
    dst = np.asarray(edge_index[1], dtype=np.int64)
    loop = np.arange(n_nodes, dtype=np.int64)
    src_all = np.concatenate([src, loop])
    dst_all = np.concatenate([dst, loop])

    deg = np.bincount(dst_all, minlength=n_nodes).astype(np.float64)
    dis = np.where(deg > 0, 1.0 / np.sqrt(deg), 0.0)
    norm = (dis[src_all] * dis[dst_all]).astype(np.float32)

    core = dst_all // nloc
    dloc = dst_all - core * nloc
    w = dloc >> 7
    drel = (dloc & 127).astype(np.float32)

    # layer 1: gather x rows; ranges of RNG node rows.  Self-loops are
    # excluded — they are fed from the resident local x window on device
    # (their diagonal S carries dinv = 1/deg).
    ne = len(src)
    r1 = src // RNG
    nr1 = int(src_all.max()) // RNG + 1
    s1 = _mkstream(
        n_cores, wn, core[:ne], w[:ne],
        gsub=r1, n_gsub=nr1, queue_of_gsub=list(range(nr1)), nq=nr1,
        idxval=src - r1 * RNG, drel=drel[:ne], nrm=norm[:ne],
    )
    s1["par"] = None

    # per-window dinv columns for the layer-1 self-loop diagonal tiles
    dinv = (dis * dis).astype(np.float32)  # 1/deg
    npad = wn * 128
    dinvw = []
    for c in range(n_cores):
        dpad = np.zeros(npad, dtype=np.float32)
        dpad[:nloc] = dinv[c * nloc : (c + 1) * nloc]
        dinvw.append(dpad.reshape(wn, 128).T.copy())  # [128, wn]

    # layer 2: gather acc1 pair rows (two 64-wide nodes per 256B row);
    # subgroup = (range, parity) so tiles are single-half
    p = src_all >> 1
    r2 = p // RNG
    nr2 = int(r2.max()) + 1
    par = (src_all & 1).astype(np.int64)
    g2 = r2 * 2 + par
    s2 = _mkstream(
        n_cores, wn, core, w,
        gsub=g2, n_gsub=nr2 * 2,
        queue_of_gsub=[g // 2 for g in range(nr2 * 2)], nq=nr2,
        idxval=p - r2 * RNG, drel=drel, nrm=norm,
    )
    s2["par"] = [g % 2 for g in range(nr2 * 2)]  # parity per subgroup

    return {
        "nloc": nloc,
        "wn": wn,
        "s1": s1,
        "s2": s2,
        "dinvw": dinvw,
        "src_all": src_all,
        "dst_all": dst_all,
        "norm_all": norm,
    }


def _build_nc(meta, n_nodes, hid, out_dim, in_dim, n_cores, rounds=1):
    nloc = meta["nloc"]
    wn = meta["wn"]
    s1, s2 = meta["s1"], meta["s2"]
    npair = n_nodes // 2

    nc = bass.Bass(num_devices=n_cores, num_swdge_queues=4)

    xtab = nc.dram_tensor("xtab", [n_nodes, in_dim], F16, kind="ExternalInput")
    xloc = nc.dram_tensor("xloc", [128, wn * 128], F16, kind="ExternalInput")
    dinvw = nc.dram_tensor("dinvw", [128, wn], F32, kind="ExternalInput")
    iotap_in = nc.dram_tensor("iotap", [128, 1], F32, kind="ExternalInput")
    idx1 = nc.dram_tensor("idx1", [128, s1["T"] * 8], I16, kind="ExternalInput")
    drel1 = nc.dram_tensor("drel1", [128, s1["T"]], F32, kind="ExternalInput")
    nrm1 = nc.dram_tensor("nrm1", [128, s1["T"]], F32, kind="ExternalInput")
    idx2 = nc.dram_tensor("idx2", [128, s2["T"] * 8], I16, kind="ExternalInput")
    drel2 = nc.dram_tensor("drel2", [128, s2["T"]], F32, kind="ExternalInput")
    nrm2 = nc.dram_tensor("nrm2", [128, s2["T"]], F32, kind="ExternalInput")
    w1 = nc.dram_tensor("w1", [in_dim, hid], F16, kind="ExternalInput")
    w2 = nc.dram_tensor("w2", [hid, out_dim], F16, kind="ExternalInput")
    b1rep = nc.dram_tensor("b1rep", [128, hid], F32, kind="ExternalInput")
    b2rep = nc.dram_tensor("b2rep", [128, out_dim], F32, kind="ExternalInput")
    iota_in = nc.dram_tensor("iota", [128, 128], F16, kind="ExternalInput")
    out = nc.dram_tensor("out", [nloc, out_dim], F32, kind="ExternalOutput")

    eq = mybir.AluOpType.is_equal
    mul = mybir.AluOpType.mult

    with tile.TileContext(nc) as tc:
        with (
            tc.tile_pool(name="const", bufs=1) as cp,
            tc.tile_pool(name="gpool", bufs=GBUFS) as gp,
            tc.tile_pool(name="spool", bufs=SBUFS) as sp,
            tc.tile_pool(name="evac", bufs=6) as ep,
            tc.tile_pool(name="ps_agg", bufs=3, space="PSUM") as pa,
            tc.tile_pool(name="ps_mm", bufs=2, space="PSUM") as pm,
            tc.tile_pool(name="ps_const", bufs=1, space="PSUM") as pc,
            tc.tile_pool(name="dram", bufs=1, space="DRAM") as dp,
        ):
            nc.gpsimd.load_library(library_config.mlp)

            # ---- resident tensors ----
            def load(name, dram, shape, dt):
                t = cp.tile(shape, dt, name=name)
                nc.sync.dma_start(out=t[:], in_=dram[:])
                return t

            idx1_t = load("idx1t", idx1, [128, s1["T"] * 8], I16)
            drel1_t = load("drel1t", drel1, [128, s1["T"]], F32)
            nrm1_t = load("nrm1t", nrm1, [128, s1["T"]], F32)
            idx2_t = load("idx2t", idx2, [128, s2["T"] * 8], I16)
            drel2_t = load("drel2t", drel2, [128, s2["T"]], F32)
            nrm2_t = load("nrm2t", nrm2, [128, s2["T"]], F32)
            w1_t = load("w1t", w1, [in_dim, hid], F16)
            w2_t = load("w2t", w2, [hid, out_dim], F16)
            b1_t = load("b1t", b1rep, [128, hid], F32)
            b2_t = load("b2t", b2rep, [128, out_dim], F32)
            iota_t = load("iotat", iota_in, [128, 128], F16)
            xloc_t = load("xloct", xloc, [128, wn * 128], F16)
            dinvw_t = load("dinvwt", dinvw, [128, wn], F32)
            iotap_t = load("iotapt", iotap_in, [128, 1], F32)

            # negated norms for the ACT-path S-build (scale = -nrm)
            nneg1_t = cp.tile([128, s1["T"]], F32, name="nneg1t")
            nc.vector.tensor_scalar_mul(out=nneg1_t[:], in0=nrm1_t[:], scalar1=-1.0)
            nneg2_t = cp.tile([128, s2["T"]], F32, name="nneg2t")
            nc.vector.tensor_scalar_mul(out=nneg2_t[:], in0=nrm2_t[:], scalar1=-1.0)

            # iota in PSUM: a non-SBUF operand keeps the DVE S-builds out of
            # 2-port perf mode, which would lock GPSIMD (SWDGE descriptor
            # rings) out of SBUF and stall the gather pipeline.
            iota_ps = pc.tile([128, 128], F32)
            nc.scalar.activation(
                out=iota_ps[:], in_=iota_t[:],
                func=mybir.ActivationFunctionType.Identity,
            )

            acc1loc = dp.tile([nloc, hid], F16, name="acc1loc")

            nidx_regs = {}

            def nidx_reg(n):
                if n not in nidx_regs:
                    nidx_regs[n] = nc.gpsimd.to_reg(n)
                return nidx_regs[n]

            def build_s(st, t, drel_t, nrm_t, nneg_t):
                """S[e, j] = norm[e] * (dst_rel[e] == j)."""
                s = sp.tile([128, 128], F16, tag="s", name="s")
                if t % ACT_NTH == ACT_NTH - 1:
                    # ACT path: relu(norm - norm*(drel-iota)^2) == norm iff eq
                    tmp = sp.tile([128, 128], F16, tag="stmp", name="stmp")
                    nc.scalar.activation(
                        out=tmp[:], in_=iota_t[:],
                        func=mybir.ActivationFunctionType.Square,
                        bias=drel_t[:, t : t + 1], scale=-1.0,
                    )
                    nc.scalar.activation(
                        out=s[:], in_=tmp[:],
                        func=mybir.ActivationFunctionType.Relu,
                        bias=nrm_t[:, t : t + 1], scale=nneg_t[:, t : t + 1],
                    )
                else:
                    nc.vector.tensor_scalar(
                        out=s[:], in0=iota_ps[:],
                        scalar1=drel_t[:, t : t + 1],
                        scalar2=nrm_t[:, t : t + 1],
                        op0=eq, op1=mul,
                    )
                return s

            def agg_pass(st, table, table_rows, idx_t, drel_t, nrm_t, nneg_t,
                         layer):
                """Chunk-pipelined gather + per-window aggregation."""
                nq = st["nq"]
                t_start = st["t_start"]
                k_wg = st["k_wg"]
                tile_first = st["tile_first"]
                par_of = st["par"]
                issued = [0] * nq
                chunks = [[] for _ in range(nq)]

                def ensure(q, tiles_needed):
                    T_q = t_start[q + 1] - t_start[q]
                    while issued[q] < tiles_needed:
                        t0 = t_start[q] + issued[q]
                        span = min(CHUNK, T_q - issued[q])
                        g = gp.tile([128, CHUNK, 128], F16, tag="g", name="g")
                        rows0 = q * RNG
                        rows1 = min(rows0 + RNG, table_rows)
                        nc.gpsimd.dma_gather(
                            g[:, 0:span, :],
                            table[rows0:rows1, :],
                            idx_t[:, t0 * 8 : (t0 + span) * 8],
                            span * 128,
                            nidx_reg(span * 128),
                            128,
                            single_packet=False,
                            queue_num=q,
                        )
                        chunks[q].append((g, t0, span))
                        issued[q] += span

                def gview(q, t):
                    while True:
                        g, t0, span = chunks[q][0]
                        if t < t0 + span:
                            return g[:, t - t0, :]
                        chunks[q].pop(0)

                for w in range(wn):
                    # consumption order: subgroups by (queue, in-queue order)
                    todo = []  # (t, q, par)
                    ktot = 1 if layer == 1 else 0  # self-tile counts in L1
                    for q in range(nq):
                        for g in st["gs_of_q"][q]:
                            kw = int(k_wg[w][g])
                            if kw == 0:
                                continue
                            ensure(q, tile_first[w][g] - t_start[q] + kw)
                            p = 0 if par_of is None else par_of[g]
                            for k in range(kw):
                                todo.append((int(tile_first[w][g]) + k, q, p))
                            ktot += kw
                    if layer == 1:
                        pt = pa.tile([in_dim, 128], F32, tag="pagg")
                        # self-loop diagonal tile from the resident local x
                        # window: S_self = (iota == p) * dinv
                        s_self = sp.tile([128, 128], F16, tag="s", name="s")
                        nc.vector.tensor_scalar(
                            out=s_self[:], in0=iota_ps[:],
                            scalar1=iotap_t[:],
                            scalar2=dinvw_t[:, w : w + 1],
                            op0=eq, op1=mul,
                        )
                        nc.tensor.matmul(
                            out=pt[:],
                            lhsT=xloc_t[:, w * 128 : (w + 1) * 128],
                            rhs=s_self[:],
                            start=True, stop=(ktot == 1),
                        )
                        ki0 = 1
                    else:
                        pt = pa.tile([hid, 128], F32, tag="pagg")
                        ki0 = 0
                    for ki, (t, q, p) in enumerate(todo, start=ki0):
                        gfull = gview(q, t)
                        if layer == 1:
                            gv = gfull[:, 0:in_dim]
                        else:
                            gv = gfull[:, p * hid : (p + 1) * hid]
                        s = build_s(st, t, drel_t, nrm_t, nneg_t)
                        nc.tensor.matmul(
                            out=pt[:], lhsT=gv, rhs=s[:],
                            start=(ki == 0), stop=(ki == ktot - 1),
                        )
                    rows = min(128, nloc - w * 128)
                    if layer == 1:
                        # acc1[w] = relu(PT.T @ W1 + b1)
                        pt_sb = ep.tile([in_dim, 128], F16, tag="ptsb")
                        nc.vector.tensor_copy(out=pt_sb[:], in_=pt[:])
                        ps2 = pm.tile([128, hid], F32, tag="ps2")
                        nc.tensor.matmul(
                            out=ps2[:], lhsT=pt_sb[:], rhs=w1_t[:],
                            start=True, stop=True,
                        )
                        zt = ep.tile([128, hid], F32, tag="zt1")
                        nc.vector.tensor_tensor(
                            out=zt[:], in0=ps2[:], in1=b1_t[:],
                            op=mybir.AluOpType.add,
                        )
                        a1 = ep.tile([128, hid], F16, tag="a1")
                        nc.vector.tensor_scalar_max(
                            out=a1[:], in0=zt[:], scalar1=0.0
                        )
                        nc.sync.dma_start(
                            out=acc1loc[w * 128 : w * 128 + rows, 0:hid],
                            in_=a1[:rows, :],
                        )
                    else:
                        # out[w] = log_softmax(P2T.T @ W2 + b2)
                        p2_sb = ep.tile([hid, 128], F16, tag="p2sb")
                        nc.vector.tensor_copy(out=p2_sb[:], in_=pt[:])
                        pso = pm.tile([128, out_dim], F32, tag="pso")
                        nc.tensor.matmul(
                            out=pso[:], lhsT=p2_sb[:], rhs=w2_t[:],
                            start=True, stop=True,
                        )
                        zt = ep.tile([128, out_dim], F32, tag="zt2")
                        nc.vector.tensor_tensor(
                            out=zt[:], in0=pso[:], in1=b2_t[:],
                            op=mybir.AluOpType.add,
                        )
                        mx = ep.tile([128, 1], F32, tag="mx")
                        nc.vector.reduce_max(
                            mx[:], zt[:], axis=mybir.AxisListType.X
                        )
                        sh = ep.tile([128, out_dim], F32, tag="sh")
                        nc.vector.tensor_tensor(
                            out=sh[:], in0=zt[:],
                            in1=mx[:].broadcast_to([128, out_dim]),
                            op=mybir.AluOpType.subtract,
                        )
                        ex = ep.tile([128, out_dim], F32, tag="ex")
                        sm = ep.tile([128, 1], F32, tag="sm")
                        nc.scalar.activation(
                            out=ex[:], in_=sh[:],
                            func=mybir.ActivationFunctionType.Exp,
                            accum_out=sm[:],
                        )
                        lnt = ep.tile([128, 1], F32, tag="lnt")
                        nc.scalar.activation(
                            out=lnt[:], in_=sm[:],
                            func=mybir.ActivationFunctionType.Ln,
                        )
                        res = ep.tile([128, out_dim], F32, tag="res")
                        nc.vector.tensor_tensor(
                            out=res[:], in0=sh[:],
                            in1=lnt[:].broadcast_to([128, out_dim]),
                            op=mybir.AluOpType.subtract,
                        )
                        nc.sync.dma_start(
                            out=out[w * 128 : w * 128 + rows, :],
                            in_=res[:rows, :],
                        )

            # ---- pipeline ----
            for rnd in range(rounds):
                h2v = dp.tile(
                    [npair, 2 * hid], F16, addr_space="Shared",
                    tag=f"h2v{rnd}", name=f"h2v{rnd}",
                )
                agg_pass(s1, xtab, n_nodes, idx1_t, drel1_t, nrm1_t, nneg1_t,
                         layer=1)
                nc.gpsimd.collective_compute(
                    "AllGather",
                    mybir.AluOpType.bypass,
                    replica_groups=[list(range(n_cores))],
                    ins=[acc1loc[:].opt()],
                    outs=[h2v[:].opt()],
                )
                agg_pass(s2, h2v, npair, idx2_t, drel2_t, nrm2_t, nneg2_t,
                         layer=2)

    _split_long_waits(nc)
    mybir.codegen_inst_isa_subclasses(nc)
    return nc


def _prepare(x, edge_index, W1, b1, W2, b2, n_nodes=N_NODES, n_cores=N_CORES):
    x = np.asarray(x, dtype=np.float32)
    W1 = np.asarray(W1, dtype=np.float32)
    b1 = np.asarray(b1, dtype=np.float32)
    W2 = np.asarray(W2, dtype=np.float32)
    b2 = np.asarray(b2, dtype=np.float32)

    in_dim = x.shape[1]
    hid = W1.shape[1]
    out_dim = W2.shape[1]

    meta = _preprocess(edge_index, n_nodes, n_cores)

    nc = _build_nc(meta, n_nodes, hid, out_dim, in_dim, n_cores)

    xtab = np.ascontiguousarray(x.astype(np.float16))
    w1h = W1.astype(np.float16)
    w2h = W2.astype(np.float16)
    b1rep = np.tile(b1[None, :], (128, 1)).astype(np.float32)
    b2rep = np.tile(b2[None, :], (128, 1)).astype(np.float32)
    iota = np.tile(np.arange(128, dtype=np.float16)[None, :], (128, 1)).copy()
    iotap = np.arange(128, dtype=np.float32).reshape(128, 1).copy()

    nloc = meta["nloc"]
    wn = meta["wn"]
    npad = wn * 128
    s1, s2 = meta["s1"], meta["s2"]
    in_maps = []
    for c in range(n_cores):
        xs = np.zeros((npad, in_dim), dtype=np.float16)
        xs[:nloc] = xtab[c * nloc : (c + 1) * nloc]
        xloc = np.ascontiguousarray(
            xs.reshape(wn, 128, in_dim).transpose(1, 0, 2).reshape(128, npad)
        )
        in_maps.append(
            {
                "xtab": xtab,
                "xloc": xloc,
                "dinvw": meta["dinvw"][c],
                "iotap": iotap,
                "idx1": s1["idx"][c],
                "drel1": s1["drel"][c],
                "nrm1": s1["nrm"][c],
                "idx2": s2["idx"][c],
                "drel2": s2["drel"][c],
                "nrm2": s2["nrm"][c],
                "w1": w1h,
                "w2": w2h,
                "b1rep": b1rep,
                "b2rep": b2rep,
                "iota": iota,
            }
        )
    return nc, in_maps


def kernel(x, edge_index, W1, b1, W2, b2):
    nc, in_maps = _prepare(x, edge_index, W1, b1, W2, b2)
    res = run_bass_kernel_spmd(nc, in_maps, core_ids=list(range(N_CORES)))
    return np.concatenate([res.results[c]["out"] for c in range(N_CORES)], axis=0)
